# revision 4
# baseline (speedup 1.0000x reference)
"""Bahdanau additive attention on TRN2, data-parallel over batch on 8 NeuronCores.

Reference computation (per batch b):
    pre[s, :]  = W1 @ hs[s, b, :] + b1 + W2 @ hidden[b, :] + b2      # (S, H)
    energy[s]  = v . tanh(pre[s, :])                                  # (S,)
    energy     = where(mask[s, b], energy, -1e10)
    attn       = softmax(energy over s)
    ctx[b, :]  = sum_s attn[s] * hs[s, b, :]                          # (H,)

Masked positions get attn == 0 exactly, so they contribute nothing to the
context. The host gathers only the unmasked sequence positions per batch
(padded to a common multiple of 128 across all 32 batches, since the SPMD
program is shared) and the device computes over NP ~ S/2 positions instead
of S. Padded slots carry an inverted-mask byte that forces their energy to
-1e10, which underflows to attn == 0. If any batch is fully masked we fall
back to NP = S with the original mask (softmax over all -1e10 is uniform,
matching the reference). The program is compiled per NP value at call time.

q[b] = W2 @ hidden[b] + b1 + b2 is an S-independent per-batch bias; the host
computes it (B x H x H matvec) and ships the 16 KB result, so the device
never sees W2/hidden and the PE's first instruction only waits on the first
W1 + hsT chunks.

Per-core layout strategy (batch-sharded, 4 batches per core):
  - hsT shard (BL, H, NP): h-major so the big matmul streams [h_in=128p, s]
    tiles; preT comes out as [h_out=128p, s] in PSUM, which makes the q bias
    a per-partition activation bias and the v-dot a K=128,M=1 matmul
    producing energy in [1, s] (free-axis softmax).
  - hsn shard (NP, BL, H): s-major for the context matmul (contract over s).
  W1 and hsT are fp16 (same 1 cycle/row PE rate as f32r, half the HBM
  traffic). W1 is pre-scaled by 32 on the host so its entries sit well
  inside fp16 normal range; the tanh activation's scale=1/32 undoes it.
  W1 streams on the vector DMA queue, hsT on sync, hsn on gpsimd, so the
  startup loads overlap.
"""

import os
import sys
from contextlib import ExitStack

import numpy as np
import ml_dtypes

# Fallback path for concourse; the axon sitecustomize normally provides it.
if "/opt/trn_rl_repo" not in sys.path:
    sys.path.append("/opt/trn_rl_repo")

import concourse.bass as bass
import concourse.bacc as bacc
import concourse.mybir as mybir
import concourse.tile as tile
from concourse import bass_utils

S, B, H = 2048, 32, 1024
NCORES = 8
BL = B // NCORES  # local batches per core
HK = H // 128     # 128-partition chunks of H
W1SCALE = 32.0    # host pre-scale on W1, undone by the tanh activation

F32 = mybir.dt.float32
F32R = mybir.dt.float32r
U8 = mybir.dt.uint8
BF16 = mybir.dt.bfloat16
FP16 = mybir.dt.float16
AF = mybir.ActivationFunctionType
AX = mybir.AxisListType

_CACHE = {}


def _blocks(np_pad):
    """Split NP into sigma-block widths of <= 512."""
    widths = []
    off = 0
    while off < np_pad:
        w = min(512, np_pad - off)
        widths.append(w)
        off += w
    return widths


def _emit(tc, aps, np_pad):
    nc = tc.nc
    ctx = aps["ctx_stack"]
    hst, hsn, w1t, vtr, qtr, masku, ctx_out = (
        aps["hst"], aps["hsn"], aps["w1t"], aps["vtr"],
        aps["qtr"], aps["masku"], aps["ctx"],
    )
    widths = _blocks(np_pad)
    offs = [sum(widths[:i]) for i in range(len(widths))]
    nblk = len(widths)
    nt = np_pad // 128  # s-chunks of 128 per batch

    def pool(name, bufs, space="SBUF"):
        return ctx.enter_context(tc.tile_pool(name=name, bufs=bufs, space=space))

    p_hst = pool("hst", 5)
    p_w1 = pool("w1", 1)
    p_small = pool("small", 1)
    p_hsn = pool("hsn", 16)
    p_tanh = pool("tanh", 3)
    p_eall = pool("eall", 2)
    p_em = pool("em", 1)
    p_mask = pool("mask", 1)
    p_ctxs = pool("ctxs", 1)
    p_attnT = pool("attnT", 2)
    p_sc = pool("sc", 2)

    pp_pre = pool("ppre", 4, space="PSUM")
    pp_en = pool("pen", 1, space="PSUM")
    pp_tr = pool("ptr", 1, space="PSUM")
    pp_ctx = pool("pctx", 2, space="PSUM")

    # ---------------- setup: small DMAs first, big streams on 3 queues ------
    ident = p_small.tile([1, 1], F32, tag="ident")
    nc.gpsimd.memset(ident[:], 1.0)

    # vt rearranged (128, HK); stays f32r for the PE energy matmul.
    vt_sb = p_small.tile([128, HK], F32R, tag="vt")
    nc.sync.dma_start(vt_sb[:], vtr[:])
    # host-computed q bias, pre-swizzled to [p, BL*k + b]
    qt_sb = p_small.tile([128, BL * HK], F32, tag="qt")
    nc.sync.dma_start(qt_sb[:], qtr[:])
    # all four batch masks in one row
    mask_all = p_mask.tile([1, BL * np_pad], U8, tag="mask")
    nc.sync.dma_start(mask_all[:], masku[:])

    # W1T streams on the vector queue, k-chunk at a time; the first hsT block
    # streams on sync in parallel, so ppre(k=0) starts after one chunk each.
    w1_sb = p_w1.tile([128, HK * H], FP16, tag="w1")
    w0 = widths[0]
    hst_first = p_hst.tile([128, HK * 512], FP16, tag="hst", name="hst_first")
    for k in range(HK):
        nc.scalar.dma_start(w1_sb[:, H * k:H * (k + 1)], w1t[128 * k:128 * (k + 1), :])
        nc.sync.dma_start(hst_first[:, w0 * k:w0 * (k + 1)], hst[0, 128 * k:128 * (k + 1), 0:w0])

    eall_t = {}
    em_t = {}
    attnT_t = {}
    rz_t = {}

    # ------------- pass 1: energies for one (batch, sigma-block) -------------
    def p1_block(b, c, first_tile=None):
        w = widths[c]
        off = offs[c]
        if c == 0:
            eall_t[b] = p_eall.tile([1, np_pad], F32, tag="eall", name=f"eall{b}")
        eall = eall_t[b]
        if first_tile is not None:
            hst_c = first_tile
        else:
            hst_c = p_hst.tile([128, HK * 512], FP16, tag="hst", name=f"hst_{b}_{c}")
            for k in range(HK):
                nc.sync.dma_start(
                    hst_c[:, w * k:w * (k + 1)],
                    hst[b, 128 * k:128 * (k + 1), off:off + w],
                )
        pen = pp_en.tile([1, 512], F32, tag="pen", name=f"pen_{b}_{c}")
        prev = None
        for m in range(HK):
            ppre = pp_pre.tile([128, 512], F32, tag="ppre", name=f"ppre_{b}_{c}_{m}")
            for k in range(HK):
                nc.tensor.matmul(
                    ppre[:, 0:w],
                    lhsT=w1_sb[:, H * k + 128 * m:H * k + 128 * m + 128],
                    rhs=hst_c[:, w * k:w * (k + 1)],
                    start=(k == 0), stop=(k == HK - 1),
                )
            # energy matmul for the previous m goes after this m's pre-block
            # so the PE never waits on the tanh.
            if prev is not None:
                pm, pth = prev
                nc.tensor.matmul(
                    pen[:, 0:w], lhsT=vt_sb[:, pm:pm + 1], rhs=pth[:, 0:w],
                    start=(pm == 0), stop=False,
                )
            th = p_tanh.tile([128, 512], F32R, tag="tanh", name=f"th_{b}_{c}_{m}")
            nc.scalar.activation(
                th[:, 0:w], ppre[:, 0:w], AF.Tanh,
                bias=qt_sb[:, BL * m + b:BL * m + b + 1], scale=1.0 / W1SCALE,
            )
            prev = (m, th)
        pm, pth = prev
        nc.tensor.matmul(
            pen[:, 0:w], lhsT=vt_sb[:, pm:pm + 1], rhs=pth[:, 0:w],
            start=False, stop=True,
        )
        nc.vector.tensor_copy(eall[:, off:off + w], pen[:, 0:w])

    # ------------- masked softmax, split so it interleaves with pass 1 ------
    def sm_pre(b):
        """DVE/ACT part: mask, max, exp, Z, 1/Z. No PE work."""
        eall = eall_t.pop(b)
        em = p_em.tile([1, np_pad], F32, tag="em", name=f"em{b}")
        # masku holds the INVERTED mask: em = minv * -1e10 + eall in one DVE op
        # (-1e10 + e rounds back to -1e10 for |e| << ulp(1e10), matching the
        # reference's where()).
        nc.vector.scalar_tensor_tensor(
            em[:], mask_all[:, b * np_pad:(b + 1) * np_pad], -1e10, eall[:],
            op0=mybir.AluOpType.mult, op1=mybir.AluOpType.add,
        )
        negmax = p_sc.tile([1, 1], F32, tag="negmax", name=f"negmax{b}")
        nc.vector.reduce_max(negmax[:], em[:], axis=AX.X, negate=True)
        zs = p_sc.tile([1, 1], F32, tag="zs", name=f"zs{b}")
        # attn (unnormalized) = exp(em - max) in place, Z accumulated alongside
        nc.scalar.activation(em[:], em[:], AF.Exp, bias=negmax[:], scale=1.0, accum_out=zs[:])
        rz = p_sc.tile([1, 1], F32, tag="rz", name=f"rz{b}")
        nc.vector.reciprocal(rz[:], zs[:])
        rz_t[b] = rz
        em_t[b] = em

    def sm_tr(b):
        """PE part: tiny transposes of attn into [s-partition, 1] layout."""
        em = em_t.pop(b)
        ptr = pp_tr.tile([128, nt], F32, tag="ptr", name=f"ptr{b}")
        for cc in range(nt):
            nc.tensor.transpose(ptr[:, cc:cc + 1], em[:, 128 * cc:128 * (cc + 1)], ident[:])
        att = p_attnT.tile([128, nt], BF16, tag="attnT", name=f"attnT{b}")
        nc.vector.tensor_copy(att[:], ptr[:])
        attnT_t[b] = att

    # ------------- pass 2: context for one batch -------------
    hsn_tiles = {}

    def p2_load(b):
        tiles = []
        for t in range(nt):
            hsn_c = p_hsn.tile([128, H], BF16, tag="hsn", name=f"hsn_{b}_{t}")
            nc.gpsimd.dma_start(hsn_c[:], hsn[128 * t:128 * (t + 1), b, :])
            tiles.append(hsn_c)
        hsn_tiles[b] = tiles

    def p2_mm(b):
        att = attnT_t.pop(b)
        rz = rz_t.pop(b)
        pc = [
            pp_ctx.tile([1, 512], F32, tag="pctx", name=f"pctx_{b}_{n}")
            for n in range(2)
        ]
        for t, hsn_c in enumerate(hsn_tiles.pop(b)):
            for n in range(2):
                nc.tensor.matmul(
                    pc[n][:],
                    lhsT=att[:, t:t + 1],
                    rhs=hsn_c[:, 512 * n:512 * (n + 1)],
                    start=(t == 0), stop=(t == nt - 1),
                )
        cs = p_ctxs.tile([1, H], F32, tag="ctxs", name=f"cs{b}")
        for n in range(2):
            nc.vector.tensor_scalar_mul(cs[:, 512 * n:512 * (n + 1)], pc[n][:], rz[:])
        nc.sync.dma_start(ctx_out[b:b + 1, :], cs[:])

    # ------------- schedule -------------
    # sm(b) pieces interleave into the middle of batch b+1's PE stream so the
    # softmax chain latency hides behind matmuls and p2_mm(b) never waits.
    p1_block(0, 0, first_tile=hst_first)
    for c in range(1, nblk):
        p1_block(0, c)
    p2_load(0)
    for b in range(1, BL):
        p1_block(b, 0)
        sm_pre(b - 1)
        if nblk > 1:
            p1_block(b, 1)
        sm_tr(b - 1)
        for c in range(2, nblk):
            p1_block(b, c)
        p2_mm(b - 1)
        p2_load(b)
    sm_pre(BL - 1)
    sm_tr(BL - 1)
    p2_mm(BL - 1)


def build_program(np_pad):
    if np_pad in _CACHE:
        return _CACHE[np_pad]
    nc = bacc.Bacc("TRN2", target_bir_lowering=False, debug=False, enable_asserts=False)
    aps = {
        "hst": nc.dram_tensor("hst", (BL, H, np_pad), FP16, kind="ExternalInput").ap(),
        "hsn": nc.dram_tensor("hsn", (np_pad, BL, H), BF16, kind="ExternalInput").ap(),
        "w1t": nc.dram_tensor("w1t", (H, H), FP16, kind="ExternalInput").ap(),
        "vtr": nc.dram_tensor("vtr", (128, HK), F32R, kind="ExternalInput").ap(),
        "qtr": nc.dram_tensor("qtr", (128, BL * HK), F32, kind="ExternalInput").ap(),
        "masku": nc.dram_tensor("masku", (1, BL * np_pad), U8, kind="ExternalInput").ap(),
        "ctx": nc.dram_tensor("ctx", (BL, H), F32, kind="ExternalOutput").ap(),
    }
    with tile.TileContext(nc) as tc:
        with ExitStack() as stack:
            aps["ctx_stack"] = stack
            _emit(tc, aps, np_pad)
    nc.compile()
    _CACHE[np_pad] = nc
    return nc


def plan_from_masks(masks):
    """Gather plan: per-batch unmasked indices padded to a common NP."""
    masks = np.asarray(masks).astype(bool)  # (S, B)
    counts = masks.sum(axis=0)
    if counts.min() == 0:
        # Fully-masked batch: fall back to the ungathered layout with the
        # original mask (softmax over all -1e10 is uniform like the reference).
        idx = [np.arange(S)] * B
        valid = masks.T.copy()
        return idx, S, valid
    np_pad = min(S, max(128, int(-(-counts.max() // 128)) * 128))
    valid = np.zeros((B, np_pad), dtype=bool)
    idx = []
    for b in range(B):
        i = np.flatnonzero(masks[:, b])
        valid[b, :len(i)] = True
        idx.append(np.pad(i, (0, np_pad - len(i))))
    return idx, np_pad, valid


def prep_in_maps(inputs):
    hidden = np.ascontiguousarray(np.asarray(inputs["hidden"], dtype=np.float32))
    hs = np.ascontiguousarray(np.asarray(inputs["hidden_sequence"], dtype=np.float32))
    masks = np.asarray(inputs["input_masks"])
    idx, np_pad, valid = plan_from_masks(masks)
    w1t = np.ascontiguousarray(
        (np.asarray(inputs["W1"], dtype=np.float32).T * W1SCALE).astype(np.float16)
    )
    W2 = np.asarray(inputs["W2"], dtype=np.float32)
    b1 = np.asarray(inputs["b1"], dtype=np.float32)
    b2 = np.asarray(inputs["b2"], dtype=np.float32)
    v = np.asarray(inputs["v"], dtype=np.float32)
    # q[b] = W2 @ hidden[b] + b1 + b2: S-independent per-batch bias, host-side
    q = hidden[0] @ W2.T + b1 + b2  # (B, H)
    vtr = np.ascontiguousarray(v.reshape(HK, 128).T)
    in_maps = []
    for ci in range(NCORES):
        g = slice(BL * ci, BL * (ci + 1))
        gb = range(BL * ci, BL * (ci + 1))
        # gathered per-batch sequences: gath[b_local] is (np_pad, H)
        gath = [hs[idx[b], b, :] for b in gb]
        # qtr[p, BL*k + b] = q[b, 128k + p]
        qtr = np.ascontiguousarray(
            q[g].T.reshape(HK, 128, BL).transpose(1, 0, 2).reshape(128, HK * BL)
        )
        hst_c = np.stack([gb_.T for gb_ in gath])  # (BL, H, np_pad)
        hsn_c = np.stack(gath, axis=1)  # (np_pad, BL, H)
        mask_c = ~valid[g]  # inverted: 1 forces -1e10
        in_maps.append({
            "hst": np.ascontiguousarray(hst_c.astype(np.float16)),
            "hsn": np.ascontiguousarray(hsn_c.astype(ml_dtypes.bfloat16)),
            "w1t": w1t,
            "vtr": vtr,
            "qtr": qtr,
            "masku": np.ascontiguousarray(mask_c).astype(np.uint8).reshape(1, BL * np_pad),
        })
    return in_maps, np_pad


def kernel(**inputs):
    in_maps, np_pad = prep_in_maps(inputs)
    nc = build_program(np_pad)
    res = bass_utils.run_bass_kernel_spmd(nc, in_maps, list(range(NCORES)))
    out = np.concatenate([res.results[i]["ctx"] for i in range(NCORES)], axis=0)
    return out[None].astype(np.float32)


if __name__ == "__main__":
    build_program(1024)
    print("program built OK")


# revision 5
# speedup vs baseline: 1.1341x; 1.1341x over previous
"""Bahdanau additive attention on TRN2, data-parallel over batch on 8 NeuronCores.

Reference computation (per batch b):
    pre[s, :]  = W1 @ hs[s, b, :] + b1 + W2 @ hidden[b, :] + b2      # (S, H)
    energy[s]  = v . tanh(pre[s, :])                                  # (S,)
    energy     = where(mask[s, b], energy, -1e10)
    attn       = softmax(energy over s)
    ctx[b, :]  = sum_s attn[s] * hs[s, b, :]                          # (H,)

Masked positions get attn == 0 exactly, so they contribute nothing to the
context. The host gathers only the unmasked sequence positions per batch
(padded to a common multiple of 128 across all 32 batches, since the SPMD
program is shared) and the device computes over NP ~ S/2 positions instead
of S. Padded slots carry an inverted-mask byte that forces their energy to
-1e10, which underflows to attn == 0. If any batch is fully masked we fall
back to NP = S with the original mask (softmax over all -1e10 is uniform,
matching the reference). The program is compiled per NP value at call time.

q[b] = W2 @ hidden[b] + b1 + b2 is an S-independent per-batch bias; the host
computes it (B x H x H matvec) and ships the 16 KB result, so the device
never sees W2/hidden and the PE's first instruction only waits on the first
W1 + hsT chunks.

Per-core layout strategy (batch-sharded, 4 batches per core):
  - hsT shard (BL, H, NP): h-major so the big matmul streams [h_in=128p, s]
    tiles; preT comes out as [h_out=128p, s] in PSUM, which makes the q bias
    a per-partition activation bias and the v-dot a K=128,M=1 matmul
    producing energy in [1, s] (free-axis softmax).
  - hsn shard (NP, BL, H): s-major for the context matmul (contract over s).
  W1 and hsT are fp16 (same 1 cycle/row PE rate as f32r, half the HBM
  traffic). W1 is pre-scaled by 32 on the host so its entries sit well
  inside fp16 normal range; the tanh activation's scale=1/32 undoes it.
  W1 streams on the vector DMA queue, hsT on sync, hsn on gpsimd, so the
  startup loads overlap.
"""

import os
import sys
from contextlib import ExitStack

import numpy as np
import ml_dtypes

# Fallback path for concourse; the axon sitecustomize normally provides it.
if "/opt/trn_rl_repo" not in sys.path:
    sys.path.append("/opt/trn_rl_repo")

import concourse.bass as bass
import concourse.bacc as bacc
import concourse.mybir as mybir
import concourse.tile as tile
from concourse import bass_utils

S, B, H = 2048, 32, 1024
NCORES = 8
BL = B // NCORES  # local batches per core
HK = H // 128     # 128-partition chunks of H
W1SCALE = 32.0    # host pre-scale on W1, undone by the tanh activation

F32 = mybir.dt.float32
F32R = mybir.dt.float32r
U8 = mybir.dt.uint8
BF16 = mybir.dt.bfloat16
FP16 = mybir.dt.float16
AF = mybir.ActivationFunctionType
AX = mybir.AxisListType

_CACHE = {}


def _blocks(np_pad):
    """Split NP into sigma-block widths of <= 512."""
    widths = []
    off = 0
    while off < np_pad:
        w = min(512, np_pad - off)
        widths.append(w)
        off += w
    return widths


def _emit(tc, aps, np_pad):
    nc = tc.nc
    ctx = aps["ctx_stack"]
    hst, hsn, w1t, vtr, qtr, masku, ctx_out, esc = (
        aps["hst"], aps["hsn"], aps["w1t"], aps["vtr"],
        aps["qtr"], aps["masku"], aps["ctx"], aps["esc"],
    )
    widths = _blocks(np_pad)
    offs = [sum(widths[:i]) for i in range(len(widths))]
    nblk = len(widths)
    nt = np_pad // 128  # s-chunks of 128 per batch

    def pool(name, bufs, space="SBUF"):
        return ctx.enter_context(tc.tile_pool(name=name, bufs=bufs, space=space))

    p_hst = pool("hst", 5)
    p_w1 = pool("w1", 1)
    p_small = pool("small", 1)
    p_hsn = pool("hsn", 16)
    p_tanh = pool("tanh", 3)
    p_eall = pool("eall", 2)
    p_em = pool("em", 1)
    p_mask = pool("mask", 1)
    p_ctxs = pool("ctxs", 1)
    p_attnT = pool("attnT", 2)
    p_em16 = pool("em16", 2)
    p_sc = pool("sc", 2)

    pp_pre = pool("ppre", 4, space="PSUM")
    pp_en = pool("pen", 1, space="PSUM")
    pp_ctx = pool("pctx", 2, space="PSUM")

    # ---------------- setup: small DMAs first, big streams on 3 queues ------
    # vt rearranged (128, HK); stays f32r for the PE energy matmul.
    vt_sb = p_small.tile([128, HK], BF16, tag="vt")
    nc.sync.dma_start(vt_sb[:], vtr[:])
    # host-computed q bias, pre-swizzled to [p, BL*k + b]
    qt_sb = p_small.tile([128, BL * HK], F32, tag="qt")
    nc.sync.dma_start(qt_sb[:], qtr[:])
    # all four batch masks in one row
    mask_all = p_mask.tile([1, BL * np_pad], U8, tag="mask")
    nc.sync.dma_start(mask_all[:], masku[:])

    # W1T streams on the vector queue, k-chunk at a time; the first hsT block
    # streams on sync in parallel, so ppre(k=0) starts after one chunk each.
    w1_sb = p_w1.tile([128, HK * H], FP16, tag="w1")
    w0 = widths[0]
    hst_first = p_hst.tile([128, HK * 512], FP16, tag="hst", name="hst_first")
    for k in range(HK):
        nc.sync.dma_start(w1_sb[:, H * k:H * (k + 1)], w1t[128 * k:128 * (k + 1), :])
        nc.sync.dma_start(hst_first[:, w0 * k:w0 * (k + 1)], hst[0, 128 * k:128 * (k + 1), 0:w0])

    eall_t = {}
    attnT_t = {}
    rz_t = {}

    # ------------- pass 1: energies for one (batch, sigma-block) -------------
    def p1_block(b, c, first_tile=None):
        w = widths[c]
        off = offs[c]
        if c == 0:
            eall_t[b] = p_eall.tile([1, np_pad], F32, tag="eall", name=f"eall{b}")
        eall = eall_t[b]
        if first_tile is not None:
            hst_c = first_tile
        else:
            hst_c = p_hst.tile([128, HK * 512], FP16, tag="hst", name=f"hst_{b}_{c}")
            for k in range(HK):
                nc.sync.dma_start(
                    hst_c[:, w * k:w * (k + 1)],
                    hst[b, 128 * k:128 * (k + 1), off:off + w],
                )
        pen = pp_en.tile([1, 512], F32, tag="pen", name=f"pen_{b}_{c}")
        prev = None
        for m in range(HK):
            ppre = pp_pre.tile([128, 512], F32, tag="ppre", name=f"ppre_{b}_{c}_{m}")
            for k in range(HK):
                nc.tensor.matmul(
                    ppre[:, 0:w],
                    lhsT=w1_sb[:, H * k + 128 * m:H * k + 128 * m + 128],
                    rhs=hst_c[:, w * k:w * (k + 1)],
                    start=(k == 0), stop=(k == HK - 1),
                )
            # energy matmul for the previous m goes after this m's pre-block
            # so the PE never waits on the tanh.
            if prev is not None:
                pm, pth = prev
                nc.tensor.matmul(
                    pen[:, 0:w], lhsT=vt_sb[:, pm:pm + 1], rhs=pth[:, 0:w],
                    start=(pm == 0), stop=False,
                )
            th = p_tanh.tile([128, 512], BF16, tag="tanh", name=f"th_{b}_{c}_{m}")
            nc.scalar.activation(
                th[:, 0:w], ppre[:, 0:w], AF.Tanh,
                bias=qt_sb[:, BL * m + b:BL * m + b + 1], scale=1.0 / W1SCALE,
            )
            prev = (m, th)
        pm, pth = prev
        nc.tensor.matmul(
            pen[:, 0:w], lhsT=vt_sb[:, pm:pm + 1], rhs=pth[:, 0:w],
            start=False, stop=True,
        )
        nc.vector.tensor_copy(eall[:, off:off + w], pen[:, 0:w])

    # ------------- masked softmax, split so it interleaves with pass 1 ------
    def sm_pre(b):
        """DVE/ACT part: mask, max, exp, Z, 1/Z. No PE work."""
        eall = eall_t.pop(b)
        em = p_em.tile([1, np_pad], F32, tag="em", name=f"em{b}")
        # masku holds the INVERTED mask: em = minv * -1e10 + eall in one DVE op
        # (-1e10 + e rounds back to -1e10 for |e| << ulp(1e10), matching the
        # reference's where()).
        nc.vector.scalar_tensor_tensor(
            em[:], mask_all[:, b * np_pad:(b + 1) * np_pad], -1e10, eall[:],
            op0=mybir.AluOpType.mult, op1=mybir.AluOpType.add,
        )
        negmax = p_sc.tile([1, 1], F32, tag="negmax", name=f"negmax{b}")
        nc.vector.reduce_max(negmax[:], em[:], axis=AX.X, negate=True)
        zs = p_sc.tile([1, 1], F32, tag="zs", name=f"zs{b}")
        # attn (unnormalized) = exp(em - max) in place, Z accumulated alongside
        nc.scalar.activation(em[:], em[:], AF.Exp, bias=negmax[:], scale=1.0, accum_out=zs[:])
        rz = p_sc.tile([1, 1], F32, tag="rz", name=f"rz{b}")
        nc.vector.reciprocal(rz[:], zs[:])
        rz_t[b] = rz
        # transpose attn to [s-partition, nt] with two DMAs through scratch
        # DRAM (same ring, so the write lands before the strided read-back);
        # keeps the PE out of the softmax entirely.
        em16 = p_em16.tile([1, np_pad], BF16, tag="em16", name=f"em16_{b}")
        nc.vector.tensor_copy(em16[:], em[:])
        nc.sync.dma_start(esc[b:b + 1, :], em16[:])
        att = p_attnT.tile([128, nt], BF16, tag="attnT", name=f"attnT{b}")
        nc.sync.dma_start(att[:], esc[b:b + 1, :].rearrange("o (t p) -> (o p) t", p=128))
        attnT_t[b] = att

    # ------------- pass 2: context for one batch -------------
    hsn_tiles = {}

    def p2_load(b):
        tiles = []
        for t in range(nt):
            hsn_c = p_hsn.tile([128, H], BF16, tag="hsn", name=f"hsn_{b}_{t}")
            nc.gpsimd.dma_start(hsn_c[:], hsn[128 * t:128 * (t + 1), b, :])
            tiles.append(hsn_c)
        hsn_tiles[b] = tiles

    def p2_mm(b):
        att = attnT_t.pop(b)
        rz = rz_t.pop(b)
        pc = [
            pp_ctx.tile([1, 512], F32, tag="pctx", name=f"pctx_{b}_{n}")
            for n in range(2)
        ]
        for t, hsn_c in enumerate(hsn_tiles.pop(b)):
            for n in range(2):
                nc.tensor.matmul(
                    pc[n][:],
                    lhsT=att[:, t:t + 1],
                    rhs=hsn_c[:, 512 * n:512 * (n + 1)],
                    start=(t == 0), stop=(t == nt - 1),
                )
        cs = p_ctxs.tile([1, H], F32, tag="ctxs", name=f"cs{b}")
        for n in range(2):
            nc.vector.tensor_scalar_mul(cs[:, 512 * n:512 * (n + 1)], pc[n][:], rz[:])
        nc.sync.dma_start(ctx_out[b:b + 1, :], cs[:])

    # ------------- schedule -------------
    # sm(b) pieces interleave into the middle of batch b+1's PE stream so the
    # softmax chain latency hides behind matmuls and p2_mm(b) never waits.
    p1_block(0, 0, first_tile=hst_first)
    for c in range(1, nblk):
        p1_block(0, c)
    p2_load(0)
    for b in range(1, BL):
        p1_block(b, 0)
        sm_pre(b - 1)
        for c in range(1, nblk):
            p1_block(b, c)
        p2_mm(b - 1)
        p2_load(b)
    sm_pre(BL - 1)
    p2_mm(BL - 1)


def build_program(np_pad):
    if np_pad in _CACHE:
        return _CACHE[np_pad]
    nc = bacc.Bacc("TRN2", target_bir_lowering=False, debug=False, enable_asserts=False)
    aps = {
        "hst": nc.dram_tensor("hst", (BL, H, np_pad), FP16, kind="ExternalInput").ap(),
        "hsn": nc.dram_tensor("hsn", (np_pad, BL, H), BF16, kind="ExternalInput").ap(),
        "w1t": nc.dram_tensor("w1t", (H, H), FP16, kind="ExternalInput").ap(),
        "vtr": nc.dram_tensor("vtr", (128, HK), BF16, kind="ExternalInput").ap(),
        "esc": nc.dram_tensor("esc", (BL, np_pad), BF16, kind="Internal").ap(),
        "qtr": nc.dram_tensor("qtr", (128, BL * HK), F32, kind="ExternalInput").ap(),
        "masku": nc.dram_tensor("masku", (1, BL * np_pad), U8, kind="ExternalInput").ap(),
        "ctx": nc.dram_tensor("ctx", (BL, H), F32, kind="ExternalOutput").ap(),
    }
    with tile.TileContext(nc) as tc:
        with ExitStack() as stack:
            aps["ctx_stack"] = stack
            _emit(tc, aps, np_pad)
    nc.compile()
    _CACHE[np_pad] = nc
    return nc


def plan_from_masks(masks):
    """Gather plan: per-batch unmasked indices padded to a common NP."""
    masks = np.asarray(masks).astype(bool)  # (S, B)
    counts = masks.sum(axis=0)
    if counts.min() == 0:
        # Fully-masked batch: fall back to the ungathered layout with the
        # original mask (softmax over all -1e10 is uniform like the reference).
        idx = [np.arange(S)] * B
        valid = masks.T.copy()
        return idx, S, valid
    np_pad = min(S, max(128, int(-(-counts.max() // 128)) * 128))
    valid = np.zeros((B, np_pad), dtype=bool)
    idx = []
    for b in range(B):
        i = np.flatnonzero(masks[:, b])
        valid[b, :len(i)] = True
        idx.append(np.pad(i, (0, np_pad - len(i))))
    return idx, np_pad, valid


def prep_in_maps(inputs):
    hidden = np.ascontiguousarray(np.asarray(inputs["hidden"], dtype=np.float32))
    hs = np.ascontiguousarray(np.asarray(inputs["hidden_sequence"], dtype=np.float32))
    masks = np.asarray(inputs["input_masks"])
    idx, np_pad, valid = plan_from_masks(masks)
    w1t = np.ascontiguousarray(
        (np.asarray(inputs["W1"], dtype=np.float32).T * W1SCALE).astype(np.float16)
    )
    W2 = np.asarray(inputs["W2"], dtype=np.float32)
    b1 = np.asarray(inputs["b1"], dtype=np.float32)
    b2 = np.asarray(inputs["b2"], dtype=np.float32)
    v = np.asarray(inputs["v"], dtype=np.float32)
    # q[b] = W2 @ hidden[b] + b1 + b2: S-independent per-batch bias, host-side
    q = hidden[0] @ W2.T + b1 + b2  # (B, H)
    vtr = np.ascontiguousarray(v.reshape(HK, 128).T.astype(ml_dtypes.bfloat16))
    in_maps = []
    for ci in range(NCORES):
        g = slice(BL * ci, BL * (ci + 1))
        gb = range(BL * ci, BL * (ci + 1))
        # gathered per-batch sequences: gath[b_local] is (np_pad, H)
        gath = [hs[idx[b], b, :] for b in gb]
        # qtr[p, BL*k + b] = q[b, 128k + p]
        qtr = np.ascontiguousarray(
            q[g].T.reshape(HK, 128, BL).transpose(1, 0, 2).reshape(128, HK * BL)
        )
        hst_c = np.stack([gb_.T for gb_ in gath])  # (BL, H, np_pad)
        hsn_c = np.stack(gath, axis=1)  # (np_pad, BL, H)
        mask_c = ~valid[g]  # inverted: 1 forces -1e10
        in_maps.append({
            "hst": np.ascontiguousarray(hst_c.astype(np.float16)),
            "hsn": np.ascontiguousarray(hsn_c.astype(ml_dtypes.bfloat16)),
            "w1t": w1t,
            "vtr": vtr,
            "qtr": qtr,
            "masku": np.ascontiguousarray(mask_c).astype(np.uint8).reshape(1, BL * np_pad),
        })
    return in_maps, np_pad


def kernel(**inputs):
    in_maps, np_pad = prep_in_maps(inputs)
    nc = build_program(np_pad)
    res = bass_utils.run_bass_kernel_spmd(nc, in_maps, list(range(NCORES)))
    out = np.concatenate([res.results[i]["ctx"] for i in range(NCORES)], axis=0)
    return out[None].astype(np.float32)


if __name__ == "__main__":
    build_program(1024)
    print("program built OK")


# revision 6
# speedup vs baseline: 1.1877x; 1.0472x over previous
"""Bahdanau additive attention on TRN2, data-parallel over batch on 8 NeuronCores.

Reference computation (per batch b):
    pre[s, :]  = W1 @ hs[s, b, :] + b1 + W2 @ hidden[b, :] + b2      # (S, H)
    energy[s]  = v . tanh(pre[s, :])                                  # (S,)
    energy     = where(mask[s, b], energy, -1e10)
    attn       = softmax(energy over s)
    ctx[b, :]  = sum_s attn[s] * hs[s, b, :]                          # (H,)

Masked positions get attn == 0 exactly, so they contribute nothing to the
context. The host gathers only the unmasked sequence positions per batch
(padded to a common multiple of 128 across all 32 batches, since the SPMD
program is shared) and the device computes over NP ~ S/2 positions instead
of S. Padded slots carry an inverted-mask byte that forces their energy to
-1e10, which underflows to attn == 0. If any batch is fully masked we fall
back to NP = S with the original mask (softmax over all -1e10 is uniform,
matching the reference). The program is compiled per NP value at call time.

q[b] = W2 @ hidden[b] + b1 + b2 is an S-independent per-batch bias; the host
computes it (B x H x H matvec) and ships the 16 KB result, so the device
never sees W2/hidden and the PE's first instruction only waits on the first
W1 + hsT chunks.

Per-core layout strategy (batch-sharded, 4 batches per core):
  - hsT shard (BL, H, NP): h-major so the big matmul streams [h_in=128p, s]
    tiles; preT comes out as [h_out=128p, s] in PSUM, which makes the q bias
    a per-partition activation bias and the v-dot a K=128,M=1 matmul
    producing energy in [1, s] (free-axis softmax).
  - hsn shard (NP, BL, H): s-major for the context matmul (contract over s).
  W1 and hsT are fp16 (same 1 cycle/row PE rate as f32r, half the HBM
  traffic). W1 is pre-scaled by 32 on the host so its entries sit well
  inside fp16 normal range; the tanh activation's scale=1/32 undoes it.
  W1 streams on the vector DMA queue, hsT on sync, hsn on gpsimd, so the
  startup loads overlap.
"""

import os
import sys
from contextlib import ExitStack

import numpy as np
import ml_dtypes

# Fallback path for concourse; the axon sitecustomize normally provides it.
if "/opt/trn_rl_repo" not in sys.path:
    sys.path.append("/opt/trn_rl_repo")

import concourse.bass as bass
import concourse.bacc as bacc
import concourse.mybir as mybir
import concourse.tile as tile
from concourse import bass_utils

S, B, H = 2048, 32, 1024
NCORES = 8
BL = B // NCORES  # local batches per core
HK = H // 128     # 128-partition chunks of H
W1SCALE = 32.0    # host pre-scale on W1, undone by the tanh activation

F32 = mybir.dt.float32
F32R = mybir.dt.float32r
U8 = mybir.dt.uint8
BF16 = mybir.dt.bfloat16
FP16 = mybir.dt.float16
AF = mybir.ActivationFunctionType
AX = mybir.AxisListType

_CACHE = {}


def _blocks(np_pad):
    """Split NP into sigma-block widths of <= 512."""
    widths = []
    off = 0
    while off < np_pad:
        w = min(512, np_pad - off)
        widths.append(w)
        off += w
    return widths


def _emit(tc, aps, np_pad):
    nc = tc.nc
    ctx = aps["ctx_stack"]
    hst, hsn, w1t, vtr, qtr, masku, ctx_out, esc = (
        aps["hst"], aps["hsn"], aps["w1t"], aps["vtr"],
        aps["qtr"], aps["masku"], aps["ctx"], aps["esc"],
    )
    widths = _blocks(np_pad)
    offs = [sum(widths[:i]) for i in range(len(widths))]
    nblk = len(widths)
    nt = np_pad // 128  # s-chunks of 128 per batch

    def pool(name, bufs, space="SBUF"):
        return ctx.enter_context(tc.tile_pool(name=name, bufs=bufs, space=space))

    p_hst = pool("hst", 6)
    p_w1 = pool("w1", 1)
    p_small = pool("small", 1)
    p_hsn = pool("hsn", 24)
    p_tanh = pool("tanh", 3)
    p_eall = pool("eall", 2)
    p_em = pool("em", 1)
    p_mask = pool("mask", 1)
    p_ctxs = pool("ctxs", 1)
    p_attnT = pool("attnT", 2)
    p_em16 = pool("em16", 2)
    p_sc = pool("sc", 2)

    pp_pre = pool("ppre", 4, space="PSUM")
    pp_en = pool("pen", 1, space="PSUM")
    pp_ctx = pool("pctx", 2, space="PSUM")
    pp_tr = pool("ptr", 1, space="PSUM")

    # ---------------- setup: small DMAs first, big streams on 3 queues ------
    ident = p_small.tile([1, 1], F32, tag="ident")
    nc.gpsimd.memset(ident[:], 1.0)

    # vt rearranged (128, HK), fp16 to match the tanh tiles.
    vt_sb = p_small.tile([128, HK], FP16, tag="vt")
    nc.sync.dma_start(vt_sb[:], vtr[:])
    # host-computed q bias, pre-swizzled to [p, BL*k + b]
    qt_sb = p_small.tile([128, BL * HK], F32, tag="qt")
    nc.sync.dma_start(qt_sb[:], qtr[:])
    # all four batch masks in one row
    mask_all = p_mask.tile([1, BL * np_pad], U8, tag="mask")
    nc.sync.dma_start(mask_all[:], masku[:])

    # W1T streams on the vector queue, k-chunk at a time; the first hsT block
    # streams on sync in parallel, so ppre(k=0) starts after one chunk each.
    w1_sb = p_w1.tile([128, HK * H], FP16, tag="w1")
    w0 = widths[0]
    hst_first = p_hst.tile([128, HK * 512], FP16, tag="hst", name="hst_first")
    for k in range(HK):
        nc.gpsimd.dma_start(w1_sb[:, H * k:H * (k + 1)], w1t[128 * k:128 * (k + 1), :])
        nc.sync.dma_start(hst_first[:, w0 * k:w0 * (k + 1)], hst[0, 128 * k:128 * (k + 1), 0:w0])

    eall_t = {}
    attnT_t = {}
    rz_t = {}

    # ------------- pass 1: energies for one (batch, sigma-block) -------------
    def p1_block(b, c, first_tile=None):
        w = widths[c]
        off = offs[c]
        if c == 0:
            eall_t[b] = p_eall.tile([1, np_pad], F32, tag="eall", name=f"eall{b}")
        eall = eall_t[b]
        if first_tile is not None:
            hst_c = first_tile
        else:
            hst_c = p_hst.tile([128, HK * 512], FP16, tag="hst", name=f"hst_{b}_{c}")
            for k in range(HK):
                nc.sync.dma_start(
                    hst_c[:, w * k:w * (k + 1)],
                    hst[b, 128 * k:128 * (k + 1), off:off + w],
                )
        pen = pp_en.tile([1, 512], F32, tag="pen", name=f"pen_{b}_{c}")
        prev = None
        for m in range(HK):
            ppre = pp_pre.tile([128, 512], F32, tag="ppre", name=f"ppre_{b}_{c}_{m}")
            for k in range(HK):
                nc.tensor.matmul(
                    ppre[:, 0:w],
                    lhsT=w1_sb[:, H * k + 128 * m:H * k + 128 * m + 128],
                    rhs=hst_c[:, w * k:w * (k + 1)],
                    start=(k == 0), stop=(k == HK - 1),
                )
            # energy matmul for the previous m goes after this m's pre-block
            # so the PE never waits on the tanh.
            if prev is not None:
                pm, pth = prev
                nc.tensor.matmul(
                    pen[:, 0:w], lhsT=vt_sb[:, pm:pm + 1], rhs=pth[:, 0:w],
                    start=(pm == 0), stop=False,
                )
            th = p_tanh.tile([128, 512], FP16, tag="tanh", name=f"th_{b}_{c}_{m}")
            nc.scalar.activation(
                th[:, 0:w], ppre[:, 0:w], AF.Tanh,
                bias=qt_sb[:, BL * m + b:BL * m + b + 1], scale=1.0 / W1SCALE,
            )
            prev = (m, th)
        pm, pth = prev
        nc.tensor.matmul(
            pen[:, 0:w], lhsT=vt_sb[:, pm:pm + 1], rhs=pth[:, 0:w],
            start=False, stop=True,
        )
        nc.vector.tensor_copy(eall[:, off:off + w], pen[:, 0:w])

    # ------------- masked softmax, split so it interleaves with pass 1 ------
    def sm_pre(b, pe_transpose=False):
        """DVE/ACT part: mask, max, exp, Z, 1/Z. No PE work."""
        eall = eall_t.pop(b)
        em = p_em.tile([1, np_pad], F32, tag="em", name=f"em{b}")
        # masku holds the INVERTED mask: em = minv * -1e10 + eall in one DVE op
        # (-1e10 + e rounds back to -1e10 for |e| << ulp(1e10), matching the
        # reference's where()).
        nc.vector.scalar_tensor_tensor(
            em[:], mask_all[:, b * np_pad:(b + 1) * np_pad], -1e10, eall[:],
            op0=mybir.AluOpType.mult, op1=mybir.AluOpType.add,
        )
        negmax = p_sc.tile([1, 1], F32, tag="negmax", name=f"negmax{b}")
        nc.vector.reduce_max(negmax[:], em[:], axis=AX.X, negate=True)
        zs = p_sc.tile([1, 1], F32, tag="zs", name=f"zs{b}")
        # attn (unnormalized) = exp(em - max) in place, Z accumulated alongside
        nc.scalar.activation(em[:], em[:], AF.Exp, bias=negmax[:], scale=1.0, accum_out=zs[:])
        rz = p_sc.tile([1, 1], F32, tag="rz", name=f"rz{b}")
        nc.vector.reciprocal(rz[:], zs[:])
        rz_t[b] = rz
        att = p_attnT.tile([128, nt], BF16, tag="attnT", name=f"attnT{b}")
        if pe_transpose:
            # tail batch: the PE is idle here, and the PE path is lower
            # latency than the DRAM round-trip
            ptr = pp_tr.tile([128, nt], F32, tag="ptr", name=f"ptr{b}")
            for cc in range(nt):
                nc.tensor.transpose(ptr[:, cc:cc + 1], em[:, 128 * cc:128 * (cc + 1)], ident[:])
            nc.vector.tensor_copy(att[:], ptr[:])
        else:
            # transpose attn to [s-partition, nt] with two DMAs through
            # scratch DRAM (same ring, so the write lands before the strided
            # read-back); keeps the PE out of the softmax entirely.
            em16 = p_em16.tile([1, np_pad], BF16, tag="em16", name=f"em16_{b}")
            nc.vector.tensor_copy(em16[:], em[:])
            nc.sync.dma_start(esc[b:b + 1, :], em16[:])
            nc.sync.dma_start(att[:], esc[b:b + 1, :].rearrange("o (t p) -> (o p) t", p=128))
        attnT_t[b] = att

    # ------------- pass 2: context for one batch -------------
    hsn_tiles = {}

    def p2_load(b):
        tiles = []
        for t in range(nt):
            hsn_c = p_hsn.tile([128, H], BF16, tag="hsn", name=f"hsn_{b}_{t}")
            nc.gpsimd.dma_start(hsn_c[:], hsn[128 * t:128 * (t + 1), b, :])
            tiles.append(hsn_c)
        hsn_tiles[b] = tiles

    def p2_mm(b):
        att = attnT_t.pop(b)
        rz = rz_t.pop(b)
        pc = [
            pp_ctx.tile([1, 512], F32, tag="pctx", name=f"pctx_{b}_{n}")
            for n in range(2)
        ]
        for t, hsn_c in enumerate(hsn_tiles.pop(b)):
            for n in range(2):
                nc.tensor.matmul(
                    pc[n][:],
                    lhsT=att[:, t:t + 1],
                    rhs=hsn_c[:, 512 * n:512 * (n + 1)],
                    start=(t == 0), stop=(t == nt - 1),
                )
        cs = p_ctxs.tile([1, H], F32, tag="ctxs", name=f"cs{b}")
        for n in range(2):
            nc.vector.tensor_scalar_mul(cs[:, 512 * n:512 * (n + 1)], pc[n][:], rz[:])
        nc.sync.dma_start(ctx_out[b:b + 1, :], cs[:])

    # ------------- schedule -------------
    # sm(b) pieces interleave into the middle of batch b+1's PE stream so the
    # softmax chain latency hides behind matmuls and p2_mm(b) never waits.
    p1_block(0, 0, first_tile=hst_first)
    for c in range(1, nblk):
        p1_block(0, c)
    p2_load(0)
    for b in range(1, BL):
        p1_block(b, 0)
        sm_pre(b - 1)
        for c in range(1, nblk):
            p1_block(b, c)
        if b >= 2:
            p2_mm(b - 2)
        p2_load(b)
    sm_pre(BL - 1, pe_transpose=True)
    p2_mm(BL - 2)
    p2_mm(BL - 1)


def build_program(np_pad):
    if np_pad in _CACHE:
        return _CACHE[np_pad]
    nc = bacc.Bacc("TRN2", target_bir_lowering=False, debug=False, enable_asserts=False)
    aps = {
        "hst": nc.dram_tensor("hst", (BL, H, np_pad), FP16, kind="ExternalInput").ap(),
        "hsn": nc.dram_tensor("hsn", (np_pad, BL, H), BF16, kind="ExternalInput").ap(),
        "w1t": nc.dram_tensor("w1t", (H, H), FP16, kind="ExternalInput").ap(),
        "vtr": nc.dram_tensor("vtr", (128, HK), FP16, kind="ExternalInput").ap(),
        "esc": nc.dram_tensor("esc", (BL, np_pad), BF16, kind="Internal").ap(),
        "qtr": nc.dram_tensor("qtr", (128, BL * HK), F32, kind="ExternalInput").ap(),
        "masku": nc.dram_tensor("masku", (1, BL * np_pad), U8, kind="ExternalInput").ap(),
        "ctx": nc.dram_tensor("ctx", (BL, H), F32, kind="ExternalOutput").ap(),
    }
    with tile.TileContext(nc) as tc:
        with ExitStack() as stack:
            aps["ctx_stack"] = stack
            _emit(tc, aps, np_pad)
    nc.compile()
    _CACHE[np_pad] = nc
    return nc


def plan_from_masks(masks):
    """Gather plan: per-batch unmasked indices padded to a common NP."""
    masks = np.asarray(masks).astype(bool)  # (S, B)
    counts = masks.sum(axis=0)
    if counts.min() == 0:
        # Fully-masked batch: fall back to the ungathered layout with the
        # original mask (softmax over all -1e10 is uniform like the reference).
        idx = [np.arange(S)] * B
        valid = masks.T.copy()
        return idx, S, valid
    np_pad = min(S, max(128, int(-(-counts.max() // 128)) * 128))
    valid = np.zeros((B, np_pad), dtype=bool)
    idx = []
    for b in range(B):
        i = np.flatnonzero(masks[:, b])
        valid[b, :len(i)] = True
        idx.append(np.pad(i, (0, np_pad - len(i))))
    return idx, np_pad, valid


def prep_in_maps(inputs):
    hidden = np.ascontiguousarray(np.asarray(inputs["hidden"], dtype=np.float32))
    hs = np.ascontiguousarray(np.asarray(inputs["hidden_sequence"], dtype=np.float32))
    masks = np.asarray(inputs["input_masks"])
    idx, np_pad, valid = plan_from_masks(masks)
    w1t = np.ascontiguousarray(
        (np.asarray(inputs["W1"], dtype=np.float32).T * W1SCALE).astype(np.float16)
    )
    W2 = np.asarray(inputs["W2"], dtype=np.float32)
    b1 = np.asarray(inputs["b1"], dtype=np.float32)
    b2 = np.asarray(inputs["b2"], dtype=np.float32)
    v = np.asarray(inputs["v"], dtype=np.float32)
    # q[b] = W2 @ hidden[b] + b1 + b2: S-independent per-batch bias, host-side
    q = hidden[0] @ W2.T + b1 + b2  # (B, H)
    vtr = np.ascontiguousarray(v.reshape(HK, 128).T.astype(np.float16))
    in_maps = []
    for ci in range(NCORES):
        g = slice(BL * ci, BL * (ci + 1))
        gb = range(BL * ci, BL * (ci + 1))
        # gathered per-batch sequences: gath[b_local] is (np_pad, H)
        gath = [hs[idx[b], b, :] for b in gb]
        # qtr[p, BL*k + b] = q[b, 128k + p]
        qtr = np.ascontiguousarray(
            q[g].T.reshape(HK, 128, BL).transpose(1, 0, 2).reshape(128, HK * BL)
        )
        hst_c = np.stack([gb_.T for gb_ in gath])  # (BL, H, np_pad)
        hsn_c = np.stack(gath, axis=1)  # (np_pad, BL, H)
        mask_c = ~valid[g]  # inverted: 1 forces -1e10
        in_maps.append({
            "hst": np.ascontiguousarray(hst_c.astype(np.float16)),
            "hsn": np.ascontiguousarray(hsn_c.astype(ml_dtypes.bfloat16)),
            "w1t": w1t,
            "vtr": vtr,
            "qtr": qtr,
            "masku": np.ascontiguousarray(mask_c).astype(np.uint8).reshape(1, BL * np_pad),
        })
    return in_maps, np_pad


def kernel(**inputs):
    in_maps, np_pad = prep_in_maps(inputs)
    nc = build_program(np_pad)
    res = bass_utils.run_bass_kernel_spmd(nc, in_maps, list(range(NCORES)))
    out = np.concatenate([res.results[i]["ctx"] for i in range(NCORES)], axis=0)
    return out[None].astype(np.float32)


if __name__ == "__main__":
    build_program(1024)
    print("program built OK")
